# revision 20
# baseline (speedup 1.0000x reference)
"""Distributed multi-head attention kernel for one TRN2 chip (8 NeuronCores).

Problem: y = Attention(x) with b=2, n=2048, dim=1024, heads=16, dim_head=64.

Sharding (data + tensor parallel, per the hint):
  core c:  batch b = c // 4,  head-group r = c % 4  (4 heads = 256 inner dims)
  - Each core projects q/k/v for its 4 heads from its batch's x (fp32r).
  - Attention computed head-pair-packed on the PE (scores^T layout: j on
    partitions, i on free axis); the softmax denominator comes from a
    ones-column fused into the V matmul (no max subtraction needed: scores
    are O(5) for this distribution, exp stays in range).
  - Cores exchange normalized attention outputs (bf16) with an 8-core
    AllToAll split into two halves (one per head-pair) so the first
    collective overlaps the second half of attention. Cross-batch chunks
    are zeroed via a per-core gate input so one SPMD program serves all
    8 cores; receivers sum the two batch halves.
  - After the exchange core r holds tokens [512r, 512r+512) x all 1024
    inner dims and runs the full (bf16) output projection for its quarter.
  - Host gathers the 8 (512, 1024) results into (2, 2048, 1024).
"""

import sys

if "/opt/trn_rl_repo" not in sys.path:
    sys.path.insert(0, "/opt/trn_rl_repo")

from contextlib import ExitStack

import ml_dtypes
import numpy as np

import concourse.bass as bass
from concourse import bacc
import concourse.mybir as mybir
import concourse.tile as tile
from concourse.bass_utils import run_bass_kernel_spmd
from concourse.masks import make_identity

F32 = mybir.dt.float32
F32R = mybir.dt.float32r
BF16 = mybir.dt.bfloat16
EXP = mybir.ActivationFunctionType.Exp

B, N, DIM = 2, 2048, 1024
HEADS, DH = 16, 64
INNER = HEADS * DH            # 1024
SCALE = DH ** -0.5            # 0.125
GROUP = 4                     # tensor-parallel group size (cores per batch)
IC = INNER // GROUP           # 256 inner dims per core (4 heads)
NQ = N // GROUP               # 512 output tokens per core
NEG = -1.0e30                 # additive mask bias

P = 128                       # partitions
TB = 512                      # moving-dim block (fp32 max)
NT = N // P                   # 16 token tiles
ND = DIM // P                 # 8 contraction chunks
NTB = N // TB                 # 4 token blocks

_CACHE = {}


def _mm(nc, out, lhsT, rhs, start=True, stop=True, tile_position=None):
    nc.tensor.matmul(
        out, lhsT, rhs, start=start, stop=stop, tile_position=tile_position
    )


def _build(mask_any: bool) -> bass.Bass:
    nc = bacc.Bacc()

    x = nc.declare_dram_parameter("x_b", [N, DIM], F32, False)
    wq = nc.declare_dram_parameter("wq_s", [DIM, IC], F32R, False)
    wk = nc.declare_dram_parameter("wk_s", [DIM, IC], F32R, False)
    wv = nc.declare_dram_parameter("wv_s", [DIM, IC], F32R, False)
    wo = nc.declare_dram_parameter("wo", [INNER, DIM], BF16, False)
    bo = nc.declare_dram_parameter("bo", [1, DIM], F32R, False)
    # gate[:, g] = 1.0 iff this core handles batch g (replicated down the
    # partition axis). Zeroes the cross-batch AllToAll chunks so one SPMD
    # program works on all 8 cores.
    gate = nc.declare_dram_parameter("gate", [P, 2], F32, False)
    if mask_any:
        mb = nc.declare_dram_parameter("mbias", [P, NT], F32, False)
    y = nc.declare_dram_parameter("y", [NQ, DIM], F32, True)

    with ExitStack() as ctx:
        tc = ctx.enter_context(tile.TileContext(nc))

        const = ctx.enter_context(tc.tile_pool(name="const", bufs=1))
        ident = const.tile([P, P], F32, tag="ident")
        make_identity(nc, ident[:])
        ones_f32 = const.tile([P, P], F32, tag="ones_f32")
        nc.vector.memset(ones_f32[:], 1.0)
        ones = const.tile([P, P], F32R, tag="ones")
        nc.vector.tensor_copy(ones[:], ones_f32[:])
        bo_sb = const.tile([1, DIM], F32R, tag="bo_sb")
        nc.sync.dma_start(bo_sb[:], bo.ap())
        gate_sb = const.tile([P, 2], F32, tag="gate_sb")
        nc.sync.dma_start(gate_sb[:], gate.ap())
        if mask_any:
            mb_sb = const.tile([P, NT], F32, tag="mb_sb")
            nc.sync.dma_start(mb_sb[:], mb.ap())

        # ---- persistent SBUF ----
        qk_pool = ctx.enter_context(tc.tile_pool(name="qk", bufs=4))
        q2 = [qk_pool.tile([P, N], F32R, tag="qk", name=f"q2_{hp}") for hp in range(2)]
        k2 = [qk_pool.tile([P, N], F32R, tag="qk", name=f"k2_{hp}") for hp in range(2)]
        # v_ext[h]: 16 chunks of (128 tokens, 64 v-cols + ones col)
        v_pool = ctx.enter_context(tc.tile_pool(name="vx", bufs=4))
        v_ext = [
            v_pool.tile([P, NT * (DH + 1)], F32R, tag="vx", name=f"v_ext_{h}")
            for h in range(4)
        ]
        for h in range(4):
            nc.vector.tensor_copy(v_ext[h][:, DH :: DH + 1], ones_f32[:, 0:NT])

        wo_pool = ctx.enter_context(tc.tile_pool(name="wop", bufs=ND))
        wo_sb = [
            wo_pool.tile([P, DIM], BF16, tag="wo", bufs=ND, name=f"wo_sb_{c}")
            for c in range(ND)
        ]
        for c in range(ND):
            nc.sync.dma_start(wo_sb[c][:], wo.ap()[c * P : (c + 1) * P, :])
        bo_rep = const.tile([P, DIM], F32, tag="bo_rep")

        # Two AllToAll buffers (bf16), one per head-pair: chunk j is rows
        # [128j, 128j+128) and goes to core j; chunk content = this core's
        # two heads (hp) for token quarter j%4, zeroed unless j//4 == batch.
        dram = ctx.enter_context(tc.tile_pool(name="dram", bufs=1, space="DRAM"))
        a2a_in = [
            dram.tile([8 * P, NQ], BF16, tag="a2a_in", bufs=2, name=f"a2a_in{hp}")
            for hp in range(2)
        ]
        a2a_out = [
            dram.tile([8 * P, NQ], BF16, tag="a2a_out", bufs=2, name=f"a2a_out{hp}")
            for hp in range(2)
        ]

        # ================= phase 0/1: transpose x, project q/k/v ==========
        x4 = x.ap().rearrange("(g t p) d -> g p t d", g=NTB, t=4, p=P)
        with (
            tc.tile_pool(name="ph0", bufs=1) as ph0,
            tc.tile_pool(name="ps0", bufs=4, space="PSUM") as ps0,
        ):
            xT = [
                ph0.tile([P, N], F32R, tag="xT", bufs=ND, name=f"xT_{c}")
                for c in range(ND)
            ]
            wq_sb = [
                ph0.tile([P, IC], F32R, tag="w", bufs=24, name=f"wq_sb_{c}")
                for c in range(ND)
            ]
            wk_sb = [
                ph0.tile([P, IC], F32R, tag="w", bufs=24, name=f"wk_sb_{c}")
                for c in range(ND)
            ]
            wv_sb = [
                ph0.tile([P, IC], F32R, tag="w", bufs=24, name=f"wv_sb_{c}")
                for c in range(ND)
            ]
            for c in range(ND):
                nc.sync.dma_start(wq_sb[c][:], wq.ap()[c * P : (c + 1) * P, :])
                nc.sync.dma_start(wk_sb[c][:], wk.ap()[c * P : (c + 1) * P, :])
                nc.sync.dma_start(wv_sb[c][:], wv.ap()[c * P : (c + 1) * P, :])

            # x (tokens-major) -> xT (feature-major) via PE transpose.
            # 2MB loads; per dim-chunk, 4 transposes share one PSUM bank so
            # the PSUM->SBUF copy moves (128, 512) at a time.
            for tg in range(NTB):
                xin = ph0.tile([P, 4 * DIM], F32, tag="xin", bufs=2)
                nc.sync.dma_start(
                    xin[:].rearrange("p (t d) -> p t d", t=4), x4[tg]
                )
                for c in range(ND):
                    pst = ps0.tile([P, TB], F32, tag="ps0", name="pst")
                    for tt in range(4):
                        nc.tensor.transpose(
                            pst[:, tt * P : (tt + 1) * P],
                            xin[:, tt * DIM + c * P : tt * DIM + (c + 1) * P],
                            ident[:],
                        )
                    nc.vector.tensor_copy(
                        xT[c][:, tg * TB : (tg + 1) * TB], pst[:]
                    )

            # q^T and k^T: out (128 inner, 512 tokens), accumulate over dim
            for hp in range(2):
                for tb in range(NTB):
                    psq = ps0.tile([P, TB], F32, tag="ps0", name="psq")
                    for c in range(ND):
                        _mm(
                            nc,
                            psq[:],
                            wq_sb[c][:, hp * P : (hp + 1) * P],
                            xT[c][:, tb * TB : (tb + 1) * TB],
                            start=(c == 0),
                            stop=(c == ND - 1),
                        )
                    # fold in the 1/sqrt(dh) scale on q
                    nc.vector.tensor_scalar_mul(
                        q2[hp][:, tb * TB : (tb + 1) * TB], psq[:], SCALE
                    )
                    psk = ps0.tile([P, TB], F32, tag="ps0", name="psk")
                    for c in range(ND):
                        _mm(
                            nc,
                            psk[:],
                            wk_sb[c][:, hp * P : (hp + 1) * P],
                            xT[c][:, tb * TB : (tb + 1) * TB],
                            start=(c == 0),
                            stop=(c == ND - 1),
                        )
                    nc.vector.tensor_copy(k2[hp][:, tb * TB : (tb + 1) * TB], psk[:])

            # v tokens-major: out (128 tokens, 256 inner), lhsT = xT chunk
            for t in range(NT):
                psv = ps0.tile([P, IC], F32, tag="ps0", name="psv")
                for c in range(ND):
                    _mm(
                        nc,
                        psv[:],
                        xT[c][:, t * P : (t + 1) * P],
                        wv_sb[c][:],
                        start=(c == 0),
                        stop=(c == ND - 1),
                    )
                for h in range(4):
                    nc.vector.tensor_copy(
                        v_ext[h][:, t * (DH + 1) : t * (DH + 1) + DH],
                        psv[:, h * DH : (h + 1) * DH],
                    )

        # ================= phase 2: attention (+ overlapped A2A) ==========
        with (
            tc.tile_pool(name="att", bufs=1) as att,
            tc.tile_pool(name="ps_sc", bufs=2, space="PSUM") as ps_sc,
            tc.tile_pool(name="ps_o", bufs=3, space="PSUM") as ps_o,
        ):
            for hp in range(2):
                qa, qb = q2[hp][0:DH, :], q2[hp][DH:P, :]
                ka, kb = k2[hp][0:DH, :], k2[hp][DH:P, :]
                va, vb = v_ext[2 * hp], v_ext[2 * hp + 1]
                for ib in range(NTB):
                    isl = slice(ib * TB, (ib + 1) * TB)
                    oA = ps_o.tile([P, TB], F32, tag="o", name="oA")
                    oB = ps_o.tile([P, TB], F32, tag="o", name="oB")
                    for jt in range(NT):
                        jsl = slice(jt * P, (jt + 1) * P)
                        psAB = ps_sc.tile([P, 2 * TB], F32, tag="sc", name="psAB")
                        _mm(nc, psAB[:, 0:TB], ka[:, jsl], qa[:, isl],
                            tile_position=(0, 0))
                        _mm(nc, psAB[:, TB : 2 * TB], kb[:, jsl], qb[:, isl],
                            tile_position=(DH, 0))
                        if mask_any:
                            mcol = mb_sb[:, jt : jt + 1]
                            nc.vector.tensor_scalar_add(
                                psAB[:, 0:TB], psAB[:, 0:TB], mcol
                            )
                            nc.vector.tensor_scalar_add(
                                psAB[:, TB : 2 * TB], psAB[:, TB : 2 * TB], mcol
                            )
                        e = att.tile([P, 2 * TB], F32R, tag="e", bufs=4, name="e")
                        nc.scalar.activation(e[:], psAB[:], EXP)
                        vsl = slice(jt * (DH + 1), (jt + 1) * (DH + 1))
                        _mm(nc, oA[0 : DH + 1, :], va[:, vsl], e[:, 0:TB],
                            start=(jt == 0), stop=(jt == NT - 1))
                        _mm(nc, oB[0 : DH + 1, :], vb[:, vsl], e[:, TB : 2 * TB],
                            start=(jt == 0), stop=(jt == NT - 1))

                    # normalize by the ones-column row (row DH holds sum_j E)
                    rcpA = att.tile([DH + 1, TB], F32R, tag="rcp", bufs=4, name="rcpA")
                    with nc.allow_low_precision("f32r softmax denom"):
                        nc.vector.reciprocal(
                            rcpA[DH : DH + 1, :], oA[DH : DH + 1, :]
                        )
                    rcpB = att.tile([DH + 1, TB], F32R, tag="rcp", bufs=4, name="rcpB")
                    with nc.allow_low_precision("f32r softmax denom"):
                        nc.vector.reciprocal(
                            rcpB[DH : DH + 1, :], oB[DH : DH + 1, :]
                        )
                    for (o65, rcp, half) in ((oA, rcpA, 0), (oB, rcpB, 1)):
                        rep = ps_o.tile([DH, TB], F32, tag="o", name="rep")
                        _mm(nc, rep[:], ones[DH : DH + 1, 0:DH],
                            rcp[DH : DH + 1, :], tile_position=(DH, 0))
                        rep_sb = att.tile([DH, TB], F32, tag="rep_sb", bufs=2,
                                          name="rep_sb")
                        nc.vector.tensor_copy(rep_sb[:], rep[:])
                        st = att.tile([DH, TB], BF16, tag="st", bufs=4, name="st")
                        nc.vector.tensor_mul(st[:], o65[0:DH, :], rep_sb[:])
                        for g in range(2):
                            stg = att.tile([DH, TB], BF16, tag="st", bufs=4,
                                           name="stg")
                            nc.vector.tensor_scalar_mul(
                                stg[:], st[:], gate_sb[0:DH, g : g + 1]
                            )
                            row = (4 * g + ib) * P + half * DH
                            nc.sync.dma_start(
                                a2a_in[hp][row : row + DH, :], stg[:]
                            )

                # this head-pair's exchange; the hp=0 one overlaps hp=1
                nc.gpsimd.collective_compute(
                    "AllToAll",
                    mybir.AluOpType.bypass,
                    replica_groups=[list(range(8))],
                    ins=[a2a_in[hp].opt()],
                    outs=[a2a_out[hp].opt()],
                )

        # ================= phase 3: output projection =====================
        with (
            tc.tile_pool(name="ph3", bufs=1) as ph3,
            tc.tile_pool(name="ps_f", bufs=4, space="PSUM") as ps_f,
        ):
            # bias broadcast tile (built once)
            for nb in range(DIM // TB):
                psb = ps_f.tile([P, TB], F32, tag="f", name="psb")
                _mm(nc, psb[:], ones[0:1, :], bo_sb[:, nb * TB : (nb + 1) * TB])
                nc.vector.tensor_copy(bo_rep[:, nb * TB : (nb + 1) * TB], psb[:])

            # chunk p of each a2a_out came from core p; sum the two batch
            # halves (exactly one of them is nonzero on this core).
            # aT[2i+hp] = inner dims [256i + 128hp, ...) for all 16 heads.
            aT = [
                ph3.tile([P, NQ], BF16, tag="aT", bufs=ND, name=f"aT_{c}")
                for c in range(ND)
            ]
            for c in range(ND):
                i, hp = divmod(c, 2)
                alo = ph3.tile([P, NQ], BF16, tag="alo", bufs=2, name="alo")
                nc.sync.dma_start(alo[:], a2a_out[hp][i * P : (i + 1) * P, :])
                ahi = ph3.tile([P, NQ], BF16, tag="ahi", bufs=2, name="ahi")
                nc.sync.dma_start(
                    ahi[:], a2a_out[hp][(4 + i) * P : (5 + i) * P, :]
                )
                nc.vector.tensor_add(aT[c][:], alo[:], ahi[:])

            for t in range(NQ // P):
                for nb in range(DIM // TB):
                    psf = ps_f.tile([P, TB], F32, tag="f", name="psf")
                    for c in range(ND):
                        _mm(
                            nc,
                            psf[:],
                            aT[c][:, t * P : (t + 1) * P],
                            wo_sb[c][:, nb * TB : (nb + 1) * TB],
                            start=(c == 0),
                            stop=(c == ND - 1),
                        )
                    fout = ph3.tile([P, TB], F32, tag="fout", bufs=3, name="fout")
                    nc.vector.tensor_add(
                        fout[:], psf[:], bo_rep[:, nb * TB : (nb + 1) * TB]
                    )
                    nc.sync.dma_start(
                        y.ap()[t * P : (t + 1) * P, nb * TB : (nb + 1) * TB],
                        fout[:],
                    )

    nc.compile()
    return nc


def _get_nc(mask_any: bool) -> bass.Bass:
    if mask_any not in _CACHE:
        _CACHE[mask_any] = _build(mask_any)
    return _CACHE[mask_any]


def _in_maps(x, mask, Wq, Wkv, Wo, bo, mask_any):
    maps = []
    bo2 = np.ascontiguousarray(np.asarray(bo, np.float32).reshape(1, DIM))
    wo_bf = np.ascontiguousarray(np.asarray(Wo, np.float32).astype(ml_dtypes.bfloat16))
    for c in range(8):
        b, r = divmod(c, GROUP)
        m = {
            "x_b": np.ascontiguousarray(x[b]),
            "wq_s": np.ascontiguousarray(Wq[:, r * IC : (r + 1) * IC]),
            "wk_s": np.ascontiguousarray(Wkv[:, r * IC : (r + 1) * IC]),
            "wv_s": np.ascontiguousarray(Wkv[:, INNER + r * IC : INNER + (r + 1) * IC]),
            "wo": wo_bf,
            "bo": bo2,
            "gate": np.ascontiguousarray(
                np.tile(np.array([[1.0 - b, float(b)]], np.float32), (P, 1))
            ),
        }
        if mask_any:
            mvec = np.where(mask[b], np.float32(NEG), np.float32(0.0)).astype(
                np.float32
            )
            m["mbias"] = np.ascontiguousarray(mvec.reshape(NT, P).T)
        maps.append(m)
    return maps


def run(x, mask, Wq, Wkv, Wo, bo, trace=False):
    x = np.asarray(x, np.float32)
    mask = np.asarray(mask, bool)
    Wq = np.asarray(Wq, np.float32)
    Wkv = np.asarray(Wkv, np.float32)
    Wo = np.asarray(Wo, np.float32)
    bo = np.asarray(bo, np.float32)
    mask_any = bool(mask.any())
    nc = _get_nc(mask_any)
    maps = _in_maps(x, mask, Wq, Wkv, Wo, bo, mask_any)
    res = run_bass_kernel_spmd(nc, maps, core_ids=list(range(8)), trace=trace)
    out = np.empty((B, N, DIM), np.float32)
    for c in range(8):
        b, r = divmod(c, GROUP)
        out[b, r * NQ : (r + 1) * NQ, :] = res.results[c]["y"]
    return out, res


def kernel(x, mask, Wq, Wkv, Wo, bo):
    out, _ = run(x, mask, Wq, Wkv, Wo, bo, trace=False)
    return out


# revision 28
# speedup vs baseline: 8015.4205x; 8015.4205x over previous
"""Distributed multi-head attention kernel for one TRN2 chip (8 NeuronCores).

Problem: y = Attention(x) with b=2, n=2048, dim=1024, heads=16, dim_head=64.

Sharding (data + tensor parallel, per the hint):
  core c:  batch b = c // 4,  head-group r = c % 4  (4 heads = 256 inner dims)
  - Each core projects q/k/v for its 4 heads from its batch's x (fp32r).
  - Attention computed head-pair-packed on the PE (scores^T layout: j on
    partitions, i on free axis); the softmax denominator comes from a
    ones-column fused into the V matmul (no max subtraction needed: scores
    are O(5) for this distribution, exp stays in range).
  - Cores exchange normalized attention outputs (bf16) with an 8-core
    AllToAll split into two halves (one per head-pair) so the first
    collective overlaps the second half of attention. Cross-batch chunks
    are zeroed via a per-core gate input so one SPMD program serves all
    8 cores; receivers sum the two batch halves.
  - After the exchange core r holds tokens [512r, 512r+512) x all 1024
    inner dims and runs the full (bf16) output projection for its quarter.
  - Host gathers the 8 (512, 1024) results into (2, 2048, 1024).
"""

import sys

if "/opt/trn_rl_repo" not in sys.path:
    sys.path.insert(0, "/opt/trn_rl_repo")

from contextlib import ExitStack

import ml_dtypes
import numpy as np

import concourse.bass as bass
from concourse import bacc
import concourse.mybir as mybir
import concourse.tile as tile
from concourse.bass_utils import run_bass_kernel_spmd
from concourse.masks import make_identity

F32 = mybir.dt.float32
F32R = mybir.dt.float32r
BF16 = mybir.dt.bfloat16
EXP = mybir.ActivationFunctionType.Exp

B, N, DIM = 2, 2048, 1024
HEADS, DH = 16, 64
INNER = HEADS * DH            # 1024
SCALE = DH ** -0.5            # 0.125
GROUP = 4                     # tensor-parallel group size (cores per batch)
IC = INNER // GROUP           # 256 inner dims per core (4 heads)
NQ = N // GROUP               # 512 output tokens per core
NEG = -1.0e30                 # additive mask bias

P = 128                       # partitions
TB = 512                      # moving-dim block (fp32 max)
NT = N // P                   # 16 token tiles
ND = DIM // P                 # 8 contraction chunks
NTB = N // TB                 # 4 token blocks

_CACHE = {}


def _mm(nc, out, lhsT, rhs, start=True, stop=True, tile_position=None):
    nc.tensor.matmul(
        out, lhsT, rhs, start=start, stop=stop, tile_position=tile_position
    )


def _build(mask_any: bool) -> bass.Bass:
    nc = bacc.Bacc()

    x = nc.declare_dram_parameter("x_b", [N, DIM], F32, False)
    wq = nc.declare_dram_parameter("wq_s", [DIM, IC], F32R, False)
    wk = nc.declare_dram_parameter("wk_s", [DIM, IC], F32R, False)
    wv = nc.declare_dram_parameter("wv_s", [DIM, IC], F32R, False)
    wo = nc.declare_dram_parameter("wo", [INNER, DIM], BF16, False)
    bo = nc.declare_dram_parameter("bo", [1, DIM], F32R, False)
    # gate[:, g] = 1.0 iff this core handles batch g (replicated down the
    # partition axis). Zeroes the cross-batch AllToAll chunks so one SPMD
    # program works on all 8 cores.
    gate = nc.declare_dram_parameter("gate", [P, 2], F32, False)
    if mask_any:
        mb = nc.declare_dram_parameter("mbias", [P, NT], F32, False)
    y = nc.declare_dram_parameter("y", [NQ, DIM], F32, True)

    with ExitStack() as ctx:
        tc = ctx.enter_context(tile.TileContext(nc))

        const = ctx.enter_context(tc.tile_pool(name="const", bufs=1))
        ident = const.tile([P, P], F32, tag="ident")
        make_identity(nc, ident[:])
        ones_f32 = const.tile([P, P], F32, tag="ones_f32")
        nc.vector.memset(ones_f32[:], 1.0)
        ones = const.tile([P, P], F32R, tag="ones")
        nc.vector.tensor_copy(ones[:], ones_f32[:])
        bo_sb = const.tile([1, DIM], F32R, tag="bo_sb")
        nc.sync.dma_start(bo_sb[:], bo.ap())
        gate_sb = const.tile([P, 2], F32, tag="gate_sb")
        nc.sync.dma_start(gate_sb[:], gate.ap())
        if mask_any:
            mb_sb = const.tile([P, NT], F32, tag="mb_sb")
            nc.sync.dma_start(mb_sb[:], mb.ap())

        # ---- persistent SBUF ----
        qk_pool = ctx.enter_context(tc.tile_pool(name="qk", bufs=4))
        q2 = [qk_pool.tile([P, N], F32R, tag="qk", name=f"q2_{hp}") for hp in range(2)]
        k2 = [qk_pool.tile([P, N], F32R, tag="qk", name=f"k2_{hp}") for hp in range(2)]
        # v_ext[h]: 16 chunks of (128 tokens, 64 v-cols + ones col)
        v_pool = ctx.enter_context(tc.tile_pool(name="vx", bufs=4))
        v_ext = [
            v_pool.tile([P, NT * (DH + 1)], F32R, tag="vx", name=f"v_ext_{h}")
            for h in range(4)
        ]
        for h in range(4):
            nc.vector.tensor_copy(v_ext[h][:, DH :: DH + 1], ones_f32[:, 0:NT])

        wo_pool = ctx.enter_context(tc.tile_pool(name="wop", bufs=ND))
        wo_sb = [
            wo_pool.tile([P, DIM], BF16, tag="wo", bufs=ND, name=f"wo_sb_{c}")
            for c in range(ND)
        ]
        for c in range(ND):
            nc.gpsimd.dma_start(wo_sb[c][:], wo.ap()[c * P : (c + 1) * P, :])
        bo_rep = const.tile([P, DIM], F32, tag="bo_rep")

        # Two AllToAll buffers (bf16), one per head-pair: chunk j is rows
        # [128j, 128j+128) and goes to core j; chunk content = this core's
        # two heads (hp) for token quarter j%4, zeroed unless j//4 == batch.
        dram = ctx.enter_context(tc.tile_pool(name="dram", bufs=1, space="DRAM"))
        a2a_in = [
            dram.tile([8 * P, NQ], BF16, tag="a2a_in", bufs=2, name=f"a2a_in{hp}")
            for hp in range(2)
        ]
        a2a_out = [
            dram.tile([8 * P, NQ], BF16, tag="a2a_out", bufs=2, name=f"a2a_out{hp}")
            for hp in range(2)
        ]

        # ================= phase 0/1: transpose x, project q/k/v ==========
        x4 = x.ap().rearrange("(g t p) d -> g p t d", g=NTB, t=4, p=P)
        ph0 = ctx.enter_context(tc.tile_pool(name="ph0", bufs=1))
        xT = [
            ph0.tile([P, N], F32R, tag="xT", bufs=ND, name=f"xT_{c}")
            for c in range(ND)
        ]
        wq_sb = [
            ph0.tile([P, IC], F32R, tag="w", bufs=16, name=f"wq_sb_{c}")
            for c in range(ND)
        ]
        wk_sb = [
            ph0.tile([P, IC], F32R, tag="w", bufs=16, name=f"wk_sb_{c}")
            for c in range(ND)
        ]
        for c in range(ND):
            nc.gpsimd.dma_start(wq_sb[c][:], wq.ap()[c * P : (c + 1) * P, :])
            nc.gpsimd.dma_start(wk_sb[c][:], wk.ap()[c * P : (c + 1) * P, :])

        def proj_qk(hp, pool, tag):
            for tb in range(NTB):
                psq = pool.tile([P, TB], F32, tag=tag, name="psq")
                for c in range(ND):
                    _mm(
                        nc,
                        psq[:],
                        wq_sb[c][:, hp * P : (hp + 1) * P],
                        xT[c][:, tb * TB : (tb + 1) * TB],
                        start=(c == 0),
                        stop=(c == ND - 1),
                    )
                # fold in the 1/sqrt(dh) scale on q
                nc.vector.tensor_scalar_mul(
                    q2[hp][:, tb * TB : (tb + 1) * TB], psq[:], SCALE
                )
                psk = pool.tile([P, TB], F32, tag=tag, name="psk")
                for c in range(ND):
                    _mm(
                        nc,
                        psk[:],
                        wk_sb[c][:, hp * P : (hp + 1) * P],
                        xT[c][:, tb * TB : (tb + 1) * TB],
                        start=(c == 0),
                        stop=(c == ND - 1),
                    )
                nc.vector.tensor_copy(k2[hp][:, tb * TB : (tb + 1) * TB], psk[:])

        with (
            tc.tile_pool(name="ld", bufs=1) as ld,
            tc.tile_pool(name="ps0", bufs=4, space="PSUM") as ps0,
        ):
            wv_sb = [
                ld.tile([P, IC], F32R, tag="wv", bufs=ND, name=f"wv_sb_{c}")
                for c in range(ND)
            ]
            for c in range(ND):
                nc.gpsimd.dma_start(wv_sb[c][:], wv.ap()[c * P : (c + 1) * P, :])
            # x (tokens-major) -> xT (feature-major) via PE transpose.
            for tg in range(NTB):
                xin = ld.tile([P, 4 * DIM], F32, tag="xin", bufs=2)
                nc.sync.dma_start(
                    xin[:].rearrange("p (t d) -> p t d", t=4), x4[tg]
                )
                for c in range(ND):
                    pst = ps0.tile([P, TB], F32, tag="ps0", name="pst")
                    for tt in range(4):
                        nc.tensor.transpose(
                            pst[:, tt * P : (tt + 1) * P],
                            xin[:, tt * DIM + c * P : tt * DIM + (c + 1) * P],
                            ident[:],
                        )
                    nc.vector.tensor_copy(
                        xT[c][:, tg * TB : (tg + 1) * TB], pst[:]
                    )

            proj_qk(0, ps0, "ps0")

            # v tokens-major: out (128 tokens, 256 inner), lhsT = xT chunk
            for t in range(NT):
                psv = ps0.tile([P, IC], F32, tag="ps0", name="psv")
                for c in range(ND):
                    _mm(
                        nc,
                        psv[:],
                        xT[c][:, t * P : (t + 1) * P],
                        wv_sb[c][:],
                        start=(c == 0),
                        stop=(c == ND - 1),
                    )
                for h in range(4):
                    nc.vector.tensor_copy(
                        v_ext[h][:, t * (DH + 1) : t * (DH + 1) + DH],
                        psv[:, h * DH : (h + 1) * DH],
                    )
            proj_qk(1, ps0, "ps0")

        # ================= phase 2: attention (+ overlapped A2A) ==========
        atp = ctx.enter_context(tc.tile_pool(name="atp", bufs=1))
        aT = [
            atp.tile([P, NQ], BF16, tag="aT", bufs=ND, name=f"aT_{c}")
            for c in range(ND)
        ]

        def emit_aT(hp):
            # chunk p of a2a_out[hp] came from core p; sum the two batch
            # halves (exactly one is nonzero). aT[2i+hp] = inner rows
            # [256i + 128hp, ...).
            for i in range(4):
                c = 2 * i + hp
                alo = atp.tile([P, NQ], BF16, tag="alo", bufs=2, name="alo")
                nc.sync.dma_start(alo[:], a2a_out[hp][i * P : (i + 1) * P, :])
                ahi = atp.tile([P, NQ], BF16, tag="ahi", bufs=2, name="ahi")
                nc.sync.dma_start(
                    ahi[:], a2a_out[hp][(4 + i) * P : (5 + i) * P, :]
                )
                nc.vector.tensor_add(aT[c][:], alo[:], ahi[:])

        with (
            tc.tile_pool(name="att", bufs=1) as att,
            tc.tile_pool(name="ps_sc", bufs=3, space="PSUM") as ps_sc,
            tc.tile_pool(name="ps_o", bufs=2, space="PSUM") as ps_o,
        ):
            for hp in range(2):
                qa, qb = q2[hp][0:DH, :], q2[hp][DH:P, :]
                ka, kb = k2[hp][0:DH, :], k2[hp][DH:P, :]
                va, vb = v_ext[2 * hp], v_ext[2 * hp + 1]
                for ib in range(NTB):
                    isl = slice(ib * TB, (ib + 1) * TB)
                    oA = ps_o.tile([P, TB], F32, tag="o", name="oA")
                    oB = ps_o.tile([P, TB], F32, tag="o", name="oB")
                    for jt in range(NT):
                        jsl = slice(jt * P, (jt + 1) * P)
                        psAB = ps_sc.tile([P, 2 * TB], F32, tag="sc", name="psAB")
                        _mm(nc, psAB[:, 0:TB], ka[:, jsl], qa[:, isl],
                            tile_position=(0, 0))
                        _mm(nc, psAB[:, TB : 2 * TB], kb[:, jsl], qb[:, isl],
                            tile_position=(DH, 0))
                        if mask_any:
                            mcol = mb_sb[:, jt : jt + 1]
                            nc.vector.tensor_scalar_add(
                                psAB[:, 0:TB], psAB[:, 0:TB], mcol
                            )
                            nc.vector.tensor_scalar_add(
                                psAB[:, TB : 2 * TB], psAB[:, TB : 2 * TB], mcol
                            )
                        e = att.tile([P, 2 * TB], F32R, tag="e", bufs=3, name="e")
                        nc.scalar.activation(e[:], psAB[:], EXP)
                        vsl = slice(jt * (DH + 1), (jt + 1) * (DH + 1))
                        _mm(nc, oA[0 : DH + 1, :], va[:, vsl], e[:, 0:TB],
                            start=(jt == 0), stop=(jt == NT - 1))
                        _mm(nc, oB[0 : DH + 1, :], vb[:, vsl], e[:, TB : 2 * TB],
                            start=(jt == 0), stop=(jt == NT - 1))

                    # normalize by the ones-column row (row DH holds sum_j E)
                    rcpA = att.tile([DH + 1, TB], F32R, tag="rcp", bufs=2, name="rcpA")
                    with nc.allow_low_precision("f32r softmax denom"):
                        nc.vector.reciprocal(
                            rcpA[DH : DH + 1, :], oA[DH : DH + 1, :]
                        )
                    rcpB = att.tile([DH + 1, TB], F32R, tag="rcp", bufs=2, name="rcpB")
                    with nc.allow_low_precision("f32r softmax denom"):
                        nc.vector.reciprocal(
                            rcpB[DH : DH + 1, :], oB[DH : DH + 1, :]
                        )
                    for (o65, rcp, half) in ((oA, rcpA, 0), (oB, rcpB, 1)):
                        rep = ps_sc.tile([DH, TB], F32, tag="sc", name="rep")
                        _mm(nc, rep[:], ones[DH : DH + 1, 0:DH],
                            rcp[DH : DH + 1, :], tile_position=(DH, 0))
                        rep_sb = att.tile([DH, TB], F32, tag="rep_sb", bufs=2,
                                          name="rep_sb")
                        nc.vector.tensor_copy(rep_sb[:], rep[:])
                        st = att.tile([DH, TB], BF16, tag="st", bufs=4, name="st")
                        nc.vector.tensor_mul(st[:], o65[0:DH, :], rep_sb[:])
                        for g in range(2):
                            stg = att.tile([DH, TB], BF16, tag="st", bufs=4,
                                           name="stg")
                            nc.vector.tensor_scalar_mul(
                                stg[:], st[:], gate_sb[0:DH, g : g + 1]
                            )
                            row = (4 * g + ib) * P + half * DH
                            nc.sync.dma_start(
                                a2a_in[hp][row : row + DH, :], stg[:]
                            )

                # this head-pair's exchange; the hp=0 one overlaps hp=1
                nc.gpsimd.collective_compute(
                    "AllToAll",
                    mybir.AluOpType.bypass,
                    replica_groups=[list(range(8))],
                    ins=[a2a_in[hp].opt()],
                    outs=[a2a_out[hp].opt()],
                )
                emit_aT(hp)

        # ================= phase 3: output projection =====================
        with (
            tc.tile_pool(name="ph3", bufs=1) as ph3,
            tc.tile_pool(name="ps_f", bufs=4, space="PSUM") as ps_f,
        ):
            # bias broadcast tile (built once)
            for nb in range(DIM // TB):
                psb = ps_f.tile([P, TB], F32, tag="f", name="psb")
                _mm(nc, psb[:], ones[0:1, :], bo_sb[:, nb * TB : (nb + 1) * TB])
                nc.vector.tensor_copy(bo_rep[:, nb * TB : (nb + 1) * TB], psb[:])

            for t in range(NQ // P):
                for nb in range(DIM // TB):
                    psf = ps_f.tile([P, TB], F32, tag="f", name="psf")
                    corder = [0, 2, 4, 6, 1, 3, 5, 7]
                    for ci, c in enumerate(corder):
                        _mm(
                            nc,
                            psf[:],
                            aT[c][:, t * P : (t + 1) * P],
                            wo_sb[c][:, nb * TB : (nb + 1) * TB],
                            start=(ci == 0),
                            stop=(ci == ND - 1),
                        )
                    fout = ph3.tile([P, TB], F32, tag="fout", bufs=3, name="fout")
                    nc.vector.tensor_add(
                        fout[:], psf[:], bo_rep[:, nb * TB : (nb + 1) * TB]
                    )
                    nc.sync.dma_start(
                        y.ap()[t * P : (t + 1) * P, nb * TB : (nb + 1) * TB],
                        fout[:],
                    )

    nc.compile()
    return nc


def _get_nc(mask_any: bool) -> bass.Bass:
    if mask_any not in _CACHE:
        _CACHE[mask_any] = _build(mask_any)
    return _CACHE[mask_any]


def _in_maps(x, mask, Wq, Wkv, Wo, bo, mask_any):
    maps = []
    bo2 = np.ascontiguousarray(np.asarray(bo, np.float32).reshape(1, DIM))
    wo_bf = np.ascontiguousarray(np.asarray(Wo, np.float32).astype(ml_dtypes.bfloat16))
    for c in range(8):
        b, r = divmod(c, GROUP)
        m = {
            "x_b": np.ascontiguousarray(x[b]),
            "wq_s": np.ascontiguousarray(Wq[:, r * IC : (r + 1) * IC]),
            "wk_s": np.ascontiguousarray(Wkv[:, r * IC : (r + 1) * IC]),
            "wv_s": np.ascontiguousarray(Wkv[:, INNER + r * IC : INNER + (r + 1) * IC]),
            "wo": wo_bf,
            "bo": bo2,
            "gate": np.ascontiguousarray(
                np.tile(np.array([[1.0 - b, float(b)]], np.float32), (P, 1))
            ),
        }
        if mask_any:
            mvec = np.where(mask[b], np.float32(NEG), np.float32(0.0)).astype(
                np.float32
            )
            m["mbias"] = np.ascontiguousarray(mvec.reshape(NT, P).T)
        maps.append(m)
    return maps


_RUNNER = {}


def _get_runner(mask_any: bool):
    """Build (once) a cached jax-jitted SPMD executor for the Bass module.

    Mirrors bass2jax.run_bass_via_pjrt's multi-core path, but keeps the
    jitted callable so repeated kernel() calls skip retracing/lowering.
    """
    if mask_any in _RUNNER:
        return _RUNNER[mask_any]
    import jax
    import jax.numpy as jnp
    from jax.sharding import Mesh, PartitionSpec
    from jax.experimental.shard_map import shard_map
    from concourse import bass2jax, mybir as mb

    nc = _get_nc(mask_any)
    bass2jax.install_neuronx_cc_hook()

    partition_name = (
        nc.partition_id_tensor.name if nc.partition_id_tensor else None
    )
    in_names, out_names, out_avals = [], [], []
    for alloc in nc.m.functions[0].allocations:
        if not isinstance(alloc, mybir.MemoryLocationSet):
            continue
        name = alloc.memorylocations[0].name
        if alloc.kind == "ExternalInput":
            if name != partition_name:
                in_names.append(name)
        elif alloc.kind == "ExternalOutput":
            shape = tuple(alloc.tensor_shape)
            dtype = mybir.dt.np(alloc.dtype)
            out_names.append(name)
            out_avals.append(jax.core.ShapedArray(shape, dtype))
    n_params = len(in_names)
    n_outs = len(out_avals)
    all_names = list(in_names) + list(out_names)
    if partition_name is not None:
        all_names.append(partition_name)
    donate = tuple(range(n_params, n_params + n_outs))

    def _body(*args):
        operands = list(args)
        if partition_name is not None:
            operands.append(bass2jax.partition_id_tensor())
        outs = bass2jax._bass_exec_p.bind(
            *operands,
            out_avals=tuple(out_avals),
            in_names=tuple(all_names),
            out_names=tuple(out_names),
            lowering_input_output_aliases=(),
            sim_require_finite=True,
            sim_require_nnan=True,
            nc=nc,
        )
        return tuple(outs)

    devices = jax.devices()[:8]
    mesh = Mesh(np.asarray(devices), ("core",))
    in_specs = (PartitionSpec("core"),) * (n_params + n_outs)
    out_specs = (PartitionSpec("core"),) * n_outs
    sharded = jax.jit(
        shard_map(
            _body, mesh=mesh, in_specs=in_specs, out_specs=out_specs,
            check_rep=False,
        ),
        donate_argnums=donate,
        keep_unused=True,
    )
    zero_shapes = [tuple(a.shape) for a in out_avals]
    zero_dtypes = [a.dtype for a in out_avals]

    def call(maps):
        concat_in = [
            np.concatenate([np.asarray(maps[c][nm]) for c in range(8)], axis=0)
            for nm in in_names
        ]
        concat_zeros = [
            np.zeros((8 * s[0], *s[1:]), d)
            for s, d in zip(zero_shapes, zero_dtypes)
        ]
        out_arrs = sharded(*concat_in, *concat_zeros)
        return [
            {
                nm: np.asarray(out_arrs[i]).reshape(8, *zero_shapes[i])[c]
                for i, nm in enumerate(out_names)
            }
            for c in range(8)
        ]

    _RUNNER[mask_any] = call
    return call


def run(x, mask, Wq, Wkv, Wo, bo, trace=False):
    x = np.asarray(x, np.float32)
    mask = np.asarray(mask, bool)
    Wq = np.asarray(Wq, np.float32)
    Wkv = np.asarray(Wkv, np.float32)
    Wo = np.asarray(Wo, np.float32)
    bo = np.asarray(bo, np.float32)
    mask_any = bool(mask.any())
    maps = _in_maps(x, mask, Wq, Wkv, Wo, bo, mask_any)
    results = _get_runner(mask_any)(maps)
    out = np.empty((B, N, DIM), np.float32)
    for c in range(8):
        b, r = divmod(c, GROUP)
        out[b, r * NQ : (r + 1) * NQ, :] = results[c]["y"]
    return out, results


def kernel(x, mask, Wq, Wkv, Wo, bo):
    out, _ = run(x, mask, Wq, Wkv, Wo, bo, trace=False)
    return out
